# revision 1
# baseline (speedup 1.0000x reference)
"""Trainium2 Bass kernel for nn_DecoderCell (LFADS-style decoder cell).

Strategy: pure data parallel over 8 NeuronCores (batch 32768 -> 4096/core).
On host: shard + transpose to feature-major [feat, batch] so all matmuls
contract over SBUF partitions with batch streaming in the free dimension;
weights are pre-transposed to [in, out] and replicated. On device: per-core
loop over 8 batch tiles of N=512 (one PSUM bank), controller GRU ->
co-linear/rsample -> generator GRU -> normalized-factor readout, with
f32-relaxed (tf32-like) matmuls and fp32 elementwise split across
Vector/GPSIMD engines. Activation table switches (sigmoid-set <-> exp-set)
are amortized by processing batch tiles in pairs. Matmul-feeding tensors are
declared float32r end-to-end so all DMAs are cast-free on the sync engine.
"""

import sys

sys.path.insert(0, "/opt/trn_rl_repo")

import numpy as np

import concourse.bacc as bacc
import concourse.tile as tile
import concourse.mybir as mybir

N_CORES = 8
BATCH = 32768
B_CORE = BATCH // N_CORES  # 4096
NT = 512                   # batch tile (free dim per matmul / PSUM bank)
NTILES = B_CORE // NT      # 8
PAIR = 2                   # default tiles per exp-batching group

GEN, CON, CO, FAC, CIE, EXT = 512, 256, 64, 128, 128, 16
X_DIM = 2 * CIE + EXT      # 272
H_DIM = GEN + CON + 3 * CO + EXT + FAC  # 1104
CLIP = 5.0

F32 = mybir.dt.float32
F32R = mybir.dt.float32r
AF = mybir.ActivationFunctionType
ALU = mybir.AluOpType

# h_0 row ranges (feature-major)
H_GEN = 0
H_CON = GEN
H_FAC = GEN + CON + 3 * CO + EXT  # 976
# output row ranges
O_GEN = 0
O_CON = GEN
O_MEAN = GEN + CON
O_STD = O_MEAN + CO
O_GI = O_STD + CO
O_FACT = O_GI + CO + EXT

KNOBS = {
    "p2_bufs": 3, "p1_bufs": 2,
    "io_bufs": 3, "act_bufs": 2, "tmp_bufs": 1,
    "ng_bufs": 1, "hc_bufs": 3, "gi_bufs": 3,
    "gp_nd_con": True,   # con h'=n+d on gpsimd (else DVE)
    "gp_d_con": True,    # con d=h-n on gpsimd
    "gp_gi": True,       # gi0 add on gpsimd
    "gp_rhg": True,      # half of gen r*h on gpsimd
    "gp_dg": True,       # half of gen d=h-n on gpsimd
    "gp_ndg": True,      # half of gen h'=n+d on gpsimd
    "pipelined": False,  # software-pipeline emission across pairs
    "unified_psum": False,  # co/fac matmuls allocate from p2 pool; p1 unused
    "pair": PAIR,        # tiles per exp-batching group
    "inplace": True,     # GRU update chains in-place on h tiles (no temps)
}


def ts(i, s):
    return slice(i * s, (i + 1) * s)


def build_program(has_bias: bool, repeat: int = 1):
    nc = bacc.Bacc("TRN2", target_bir_lowering=False, debug=False,
                   num_devices=N_CORES)

    xT = nc.dram_tensor("xT", [X_DIM, B_CORE], F32R, kind="ExternalInput")
    hT = nc.dram_tensor("hT", [H_DIM, B_CORE], F32R, kind="ExternalInput")
    epsT = nc.dram_tensor("epsT", [CO, B_CORE], F32, kind="ExternalInput")
    Wci_d = nc.dram_tensor("Wci", [384, 768], F32R, kind="ExternalInput")
    Wczr_d = nc.dram_tensor("Wczr", [256, 512], F32R, kind="ExternalInput")
    Wcn_d = nc.dram_tensor("Wcn", [256, 256], F32R, kind="ExternalInput")
    Wco_d = nc.dram_tensor("Wco", [256, 128], F32R, kind="ExternalInput")
    Wgi_d = nc.dram_tensor("Wgi", [80, 1536], F32R, kind="ExternalInput")
    Wgzr_d = nc.dram_tensor("Wgzr", [512, 1024], F32R, kind="ExternalInput")
    Wgn_d = nc.dram_tensor("Wgn", [512, 512], F32R, kind="ExternalInput")
    Wfac_d = nc.dram_tensor("Wfac", [512, 128], F32R, kind="ExternalInput")
    if has_bias:
        Bczr_d = nc.dram_tensor("Bczr", [128, 4], F32, kind="ExternalInput")
        Bcn_d = nc.dram_tensor("Bcn", [128, 2], F32, kind="ExternalInput")
        Bgzr_d = nc.dram_tensor("Bgzr", [128, 8], F32, kind="ExternalInput")
        Bgn_d = nc.dram_tensor("Bgn", [128, 4], F32, kind="ExternalInput")
        Bco_d = nc.dram_tensor("Bco", [64, 2], F32, kind="ExternalInput")

    outT = nc.dram_tensor("outT", [H_DIM, B_CORE], F32R,
                          kind="ExternalOutput")

    def f32(ap):
        return ap.bitcast(F32)

    import contextlib
    with tile.TileContext(nc) as tc:
        with (
            contextlib.ExitStack() as _stk,
            tc.tile_pool(name="w", bufs=1) as wp,
            tc.tile_pool(name="io", bufs=KNOBS["io_bufs"]) as io,
            tc.tile_pool(name="act", bufs=KNOBS["act_bufs"]) as act,
            tc.tile_pool(name="tmp", bufs=KNOBS["tmp_bufs"]) as tmp,
        ):
            p2 = _stk.enter_context(
                tc.tile_pool(name="p2", bufs=KNOBS["p2_bufs"], space="PSUM"))
            if KNOBS["unified_psum"]:
                p1 = None
            else:
                p1 = _stk.enter_context(
                    tc.tile_pool(name="p1", bufs=KNOBS["p1_bufs"],
                                 space="PSUM"))
            # ---- load weights once (feature-major [k, m]) ----
            def wload(dram, K, M, tag):
                kc = (K + 127) // 128
                if K % 128 == 0:
                    t = wp.tile([128, kc, M], F32R, tag=tag)
                    nc.sync.dma_start(
                        out=t,
                        in_=dram[:, :].rearrange("(c p) m -> p c m", p=128))
                else:
                    assert kc == 1
                    t = wp.tile([K, 1, M], F32R, tag=tag)
                    nc.sync.dma_start(out=t, in_=dram[:, :].unsqueeze(1))
                return t

            Wci = wload(Wci_d, 384, 768, "Wci")
            Wczr = wload(Wczr_d, 256, 512, "Wczr")
            Wcn = wload(Wcn_d, 256, 256, "Wcn")
            Wco = wload(Wco_d, 256, 128, "Wco")
            Wgi = wload(Wgi_d, 80, 1536, "Wgi")
            Wgzr = wload(Wgzr_d, 512, 1024, "Wgzr")
            Wgn = wload(Wgn_d, 512, 512, "Wgn")
            Wfac = wload(Wfac_d, 512, 128, "Wfac")
            if has_bias:
                Bczr = wp.tile([128, 4], F32, tag="Bczr")
                nc.sync.dma_start(out=Bczr, in_=Bczr_d[:, :])
                Bcn = wp.tile([128, 2], F32, tag="Bcn")
                nc.sync.dma_start(out=Bcn, in_=Bcn_d[:, :])
                Bgzr = wp.tile([128, 8], F32, tag="Bgzr")
                nc.sync.dma_start(out=Bgzr, in_=Bgzr_d[:, :])
                Bgn = wp.tile([128, 4], F32, tag="Bgn")
                nc.sync.dma_start(out=Bgn, in_=Bgn_d[:, :])
                Bco = wp.tile([64, 2], F32, tag="Bco")
                nc.sync.dma_start(out=Bco, in_=Bco_d[:, :])

            def mm_group(psum_out, pairs):
                n = len(pairs)
                for i, (lh, rh) in enumerate(pairs):
                    nc.tensor.matmul(psum_out, lh, rh,
                                     start=(i == 0), stop=(i == n - 1))

            def act_chunks(func, dst, dst_c0, psum, bias_tile, bias_c0, nch):
                if has_bias:
                    for c in range(nch):
                        nc.scalar.activation(
                            dst[:, dst_c0 + c, :], psum[:, c, :], func,
                            bias=bias_tile[:, bias_c0 + c:bias_c0 + c + 1])
                else:
                    nc.scalar.activation(dst[:, dst_c0:dst_c0 + nch, :],
                                         psum[:, 0:nch, :], func)

            # ---------------- per-tile stages ----------------
            def con_stage(j):
                jc = ts(j, NT)
                ci = io.tile([128, 2, NT], F32R, tag="ci")
                nc.sync.dma_start(
                    out=ci,
                    in_=xT[0:256, jc].rearrange("(c p) n -> p c n", p=128))
                hc = io.tile([128, 2, NT], F32R, tag="hc",
                             bufs=KNOBS["hc_bufs"])
                nc.sync.dma_start(
                    out=hc,
                    in_=hT[H_CON:H_CON + 256, jc].rearrange(
                        "(c p) n -> p c n", p=128))
                fa = io.tile([128, NT], F32R, tag="fa")
                nc.sync.dma_start(out=fa, in_=hT[H_FAC:H_FAC + 128, jc])

                x_rhs = [ci[:, 0, :], ci[:, 1, :], fa]
                h_rhs = [hc[:, 0, :], hc[:, 1, :]]

                zr = act.tile([128, 4, NT],
                              F32R if KNOBS["inplace"] else F32, tag="zr_c")
                for half in range(2):
                    pz = p2.tile([128, 2, NT], F32, tag="p2")
                    for mi in range(2):
                        m = 2 * half + mi
                        pairs = [(Wci[:, k, ts(m, 128)], x_rhs[k])
                                 for k in range(3)]
                        pairs += [(Wczr[:, k, ts(m, 128)], h_rhs[k])
                                  for k in range(2)]
                        mm_group(pz[:, mi, :], pairs)
                    act_chunks(AF.Sigmoid, zr, 2 * half, pz,
                               has_bias and Bczr, 2 * half, 2)
                z = f32(zr[:, 0:2, :]) if KNOBS["inplace"] else zr[:, 0:2, :]
                r = f32(zr[:, 2:4, :]) if KNOBS["inplace"] else zr[:, 2:4, :]

                if KNOBS["inplace"]:
                    # r*h written over r inside zr (zr is f32r for the mm)
                    nc.vector.tensor_mul(zr[:, 2:4, :], r,
                                         f32(hc[:, :, :]))
                    rh_k = [zr[:, 2 + k, :] for k in range(2)]
                else:
                    rh = tmp.tile([128, 2, NT], F32R, tag="rh")
                    nc.vector.tensor_mul(rh, r, f32(hc[:, :, :]))
                    rh_k = [rh[:, k, :] for k in range(2)]

                pn = p2.tile([128, 2, NT], F32, tag="p2")
                for mi in range(2):
                    pairs = [(Wci[:, k, ts(4 + mi, 128)], x_rhs[k])
                             for k in range(3)]
                    pairs += [(Wcn[:, k, ts(mi, 128)], rh_k[k])
                              for k in range(2)]
                    mm_group(pn[:, mi, :], pairs)
                n_t = act.tile([128, 2, NT], F32, tag="n_c")
                act_chunks(AF.Tanh, n_t, 0, pn, has_bias and Bcn, 0, 2)

                if KNOBS["inplace"]:
                    eng = nc.gpsimd if KNOBS["gp_d_con"] else nc.vector
                    eng.tensor_sub(hc[:, :, :], f32(hc[:, :, :]),
                                   n_t[:, :, :])
                    nc.vector.tensor_mul(hc[:, :, :], z, f32(hc[:, :, :]))
                    eng = nc.gpsimd if KNOBS["gp_nd_con"] else nc.vector
                    eng.tensor_add(hc[:, :, :], n_t[:, :, :],
                                   f32(hc[:, :, :]))
                else:
                    d = tmp.tile([128, 2, NT], F32, tag="d_c")
                    eng = nc.gpsimd if KNOBS["gp_d_con"] else nc.vector
                    eng.tensor_sub(d, f32(hc[:, :, :]), n_t[:, :, :])
                    nc.vector.tensor_mul(d, z, d[:, :, :])
                    eng = nc.gpsimd if KNOBS["gp_nd_con"] else nc.vector
                    eng.tensor_add(hc[:, :, :], n_t[:, :, :], d[:, :, :])
                nc.vector.tensor_scalar(hc[:, :, :], f32(hc[:, :, :]),
                                        -CLIP, CLIP,
                                        op0=ALU.max, op1=ALU.min)
                nc.sync.dma_start(out=outT[O_CON:O_CON + 256, jc].rearrange(
                    "(c p) n -> p c n", p=128), in_=hc)
                return hc

            def co_stage(j, hc):
                jc = ts(j, NT)
                if KNOBS["unified_psum"]:
                    co_ps = p2.tile([128, 2, NT], F32, tag="p2", name="co_ps")
                    pm = co_ps[0:64, 0, :]
                    pv = co_ps[0:64, 1, :]
                else:
                    pm = p1.tile([64, NT], F32, tag="p1")
                    pv = p1.tile([64, NT], F32, tag="p1")
                mm_group(pm, [(Wco[:, k, 0:64], hc[:, k, :])
                              for k in range(2)])
                mm_group(pv, [(Wco[:, k, 64:128], hc[:, k, :])
                              for k in range(2)])
                mean = io.tile([64, NT], F32R, tag="mean")
                if has_bias:
                    nc.scalar.activation(mean, pm, AF.Identity,
                                         bias=Bco[:, 0:1])
                else:
                    nc.scalar.copy(mean, pm)
                std = io.tile([64, NT], F32R, tag="std")
                nc.scalar.activation(
                    std, pv, AF.Exp, scale=0.5,
                    bias=(Bco[:, 1:2] if has_bias else 0.0))
                ep = io.tile([64, NT], F32, tag="ep")
                nc.sync.dma_start(out=ep, in_=epsT[:, jc])
                if KNOBS["inplace"]:
                    nc.vector.tensor_mul(ep, f32(std), ep[:, :])
                    t = ep
                else:
                    t = tmp.tile([64, NT], F32, tag="t")
                    nc.vector.tensor_mul(t, f32(std), ep)
                gi = io.tile([80, NT], F32R, tag="gi",
                             bufs=KNOBS["gi_bufs"])
                eng = nc.gpsimd if KNOBS["gp_gi"] else nc.vector
                eng.tensor_add(gi[0:64, :], f32(mean), t[:, :])
                nc.sync.dma_start(out=gi[64:80, :], in_=xT[256:272, jc])
                nc.sync.dma_start(out=outT[O_MEAN:O_MEAN + 64, jc], in_=mean)
                nc.sync.dma_start(out=outT[O_STD:O_STD + 64, jc], in_=std)
                nc.sync.dma_start(out=outT[O_GI:O_GI + 80, jc], in_=gi)
                return gi

            def gen_stage(j, gi):
                jc = ts(j, NT)
                hg = io.tile([128, 4, NT], F32R, tag="hg")
                nc.sync.dma_start(
                    out=hg,
                    in_=hT[H_GEN:H_GEN + 512, jc].rearrange(
                        "(c p) n -> p c n", p=128))

                zg = act.tile([128, 4, NT], F32, tag="zg")
                rg = act.tile([128, 4, NT], F32R, tag="rg")
                for half in range(4):
                    pz = p2.tile([128, 2, NT], F32, tag="p2")
                    for mi in range(2):
                        m = 2 * half + mi
                        pairs = [(Wgi[:, 0, ts(m, 128)], gi[:, :])]
                        pairs += [(Wgzr[:, k, ts(m, 128)], hg[:, k, :])
                                  for k in range(4)]
                        mm_group(pz[:, mi, :], pairs)
                    dst = zg if half < 2 else rg
                    act_chunks(AF.Sigmoid, dst, 2 * (half % 2), pz,
                               has_bias and Bgzr, 2 * half, 2)

                eng = nc.gpsimd if KNOBS["gp_rhg"] else nc.vector
                nc.vector.tensor_mul(rg[:, 0:2, :], f32(rg[:, 0:2, :]),
                                     f32(hg[:, 0:2, :]))
                eng.tensor_mul(rg[:, 2:4, :], f32(rg[:, 2:4, :]),
                               f32(hg[:, 2:4, :]))

                ng = act.tile([128, 4, NT], F32, tag="ng",
                              bufs=KNOBS["ng_bufs"])
                for half in range(2):
                    pn = p2.tile([128, 2, NT], F32, tag="p2")
                    for mi in range(2):
                        m = 2 * half + mi
                        pairs = [(Wgi[:, 0, ts(8 + m, 128)], gi[:, :])]
                        pairs += [(Wgn[:, k, ts(m, 128)], rg[:, k, :])
                                  for k in range(4)]
                        mm_group(pn[:, mi, :], pairs)
                    act_chunks(AF.Tanh, ng, 2 * half, pn, has_bias and Bgn,
                               2 * half, 2)

                if KNOBS["inplace"]:
                    eng = nc.gpsimd if KNOBS["gp_dg"] else nc.vector
                    eng.tensor_sub(hg[:, 0:2, :], f32(hg[:, 0:2, :]),
                                   ng[:, 0:2, :])
                    nc.vector.tensor_sub(hg[:, 2:4, :], f32(hg[:, 2:4, :]),
                                         ng[:, 2:4, :])
                    nc.vector.tensor_mul(hg[:, :, :], zg[:, :, :],
                                         f32(hg[:, :, :]))
                    eng = nc.gpsimd if KNOBS["gp_ndg"] else nc.vector
                    eng.tensor_add(hg[:, 0:2, :], ng[:, 0:2, :],
                                   f32(hg[:, 0:2, :]))
                    nc.vector.tensor_add(hg[:, 2:4, :], ng[:, 2:4, :],
                                         f32(hg[:, 2:4, :]))
                else:
                    dg = tmp.tile([128, 4, NT], F32, tag="dg")
                    eng = nc.gpsimd if KNOBS["gp_dg"] else nc.vector
                    eng.tensor_sub(dg[:, 0:2, :], f32(hg[:, 0:2, :]),
                                   ng[:, 0:2, :])
                    nc.vector.tensor_sub(dg[:, 2:4, :], f32(hg[:, 2:4, :]),
                                         ng[:, 2:4, :])
                    nc.vector.tensor_mul(dg[:, :, :], zg[:, :, :],
                                         dg[:, :, :])
                    eng = nc.gpsimd if KNOBS["gp_ndg"] else nc.vector
                    eng.tensor_add(hg[:, 0:2, :], ng[:, 0:2, :],
                                   dg[:, 0:2, :])
                    nc.vector.tensor_add(hg[:, 2:4, :], ng[:, 2:4, :],
                                         dg[:, 2:4, :])
                nc.vector.tensor_scalar(hg[:, :, :], f32(hg[:, :, :]),
                                        -CLIP, CLIP,
                                        op0=ALU.max, op1=ALU.min)
                nc.sync.dma_start(out=outT[O_GEN:O_GEN + 512, jc].rearrange(
                    "(c p) n -> p c n", p=128), in_=hg)

                if KNOBS["unified_psum"]:
                    pf_t = p2.tile([128, 2, NT], F32, tag="p2", name="pf_t")
                    pf = pf_t[:, 0, :]
                else:
                    pf = p1.tile([128, NT], F32, tag="p1")
                mm_group(pf, [(Wfac[:, k, :], hg[:, k, :])
                              for k in range(4)])
                fc = io.tile([128, NT], F32R, tag="fc")
                nc.scalar.copy(fc, pf)
                nc.sync.dma_start(out=outT[O_FACT:O_FACT + 128, jc], in_=fc)

            # ---- main loop: pairs of tiles, exp batched per pair ----
            pair_sz = KNOBS["pair"]
            npairs = NTILES // pair_sz

            def pair_tiles(p):
                return [p * pair_sz + i for i in range(pair_sz)]

            for _rep in range(repeat):
                if not KNOBS["pipelined"]:
                    for p in range(npairs):
                        js = pair_tiles(p)
                        hcs = {j: con_stage(j) for j in js}
                        gis = {j: co_stage(j, hcs[j]) for j in js}
                        for j in js:
                            gen_stage(j, gis[j])
                else:
                    # emit next pair's con/co before this pair's gen so
                    # non-PE engines have cross-pair work available
                    hcs, gis = {}, {}
                    for j in pair_tiles(0):
                        hcs[j] = con_stage(j)
                    for j in pair_tiles(0):
                        gis[j] = co_stage(j, hcs[j])
                    for p in range(1, npairs):
                        for j in pair_tiles(p):
                            hcs[j] = con_stage(j)
                        for j in pair_tiles(p - 1):
                            gen_stage(j, gis[j])
                        for j in pair_tiles(p):
                            gis[j] = co_stage(j, hcs[j])
                    for j in pair_tiles(npairs - 1):
                        gen_stage(j, gis[j])

    nc.compile()
    return nc


# ---------------------------------------------------------------------------
# host-side runner (cached per process)
# ---------------------------------------------------------------------------
_CACHE = {}


def _get_runner(has_bias):
    key = has_bias
    if key not in _CACHE:
        nc = build_program(has_bias)
        _CACHE[key] = _make_runner(nc)
    return _CACHE[key]


def _make_runner(nc):
    import jax
    from jax.sharding import Mesh, PartitionSpec, NamedSharding
    from jax.experimental.shard_map import shard_map
    from concourse.bass2jax import (_bass_exec_p, install_neuronx_cc_hook,
                                    partition_id_tensor)

    install_neuronx_cc_hook()
    partition_name = (nc.partition_id_tensor.name
                      if nc.partition_id_tensor else None)
    in_names, out_names, out_avals, zero_outs = [], [], [], []
    for alloc in nc.m.functions[0].allocations:
        if not isinstance(alloc, mybir.MemoryLocationSet):
            continue
        name = alloc.memorylocations[0].name
        if alloc.kind == "ExternalInput":
            if name != partition_name:
                in_names.append(name)
        elif alloc.kind == "ExternalOutput":
            shape = tuple(alloc.tensor_shape)
            dtype = mybir.dt.np(alloc.dtype)
            out_names.append(name)
            out_avals.append(jax.core.ShapedArray(shape, dtype))
            zero_outs.append(np.zeros(shape, dtype))
    all_in = in_names + out_names
    if partition_name is not None:
        all_in.append(partition_name)

    def _body(*args):
        operands = list(args)
        if partition_name is not None:
            operands.append(partition_id_tensor())
        return tuple(_bass_exec_p.bind(
            *operands, out_avals=tuple(out_avals), in_names=tuple(all_in),
            out_names=tuple(out_names),
            lowering_input_output_aliases=(),
            sim_require_finite=True, sim_require_nnan=True, nc=nc))

    devices = jax.devices()[:N_CORES]
    mesh = Mesh(np.asarray(devices), ("core",))
    nin = len(in_names)
    fn = jax.jit(
        shard_map(_body, mesh=mesh,
                  in_specs=(PartitionSpec("core"),) * (nin + len(out_names)),
                  out_specs=(PartitionSpec("core"),) * len(out_names),
                  check_rep=False),
        keep_unused=True)
    sharding = NamedSharding(mesh, PartitionSpec("core"))

    class R:
        pass

    r = R()
    r.jax = jax
    r.fn = fn
    r.sharding = sharding
    r.in_names = in_names
    r.out_names = out_names
    r.out_avals = out_avals
    r.zero_outs = zero_outs
    return r


def _prep_inputs(input, h_0, eps, gen_w_ih, gen_w_hh, con_w_ih, con_w_hh,
                 co_w, fac_w, biases):
    f = np.float32
    input = np.asarray(input, f)
    h_0 = np.asarray(h_0, f)
    eps = np.asarray(eps, f)
    gen_w_ih = np.asarray(gen_w_ih, f)
    gen_w_hh = np.asarray(gen_w_hh, f)
    con_w_ih = np.asarray(con_w_ih, f)
    con_w_hh = np.asarray(con_w_hh, f)
    co_w = np.asarray(co_w, f)
    fac_w = np.asarray(fac_w, f)

    norm = np.maximum(np.linalg.norm(fac_w, axis=1, keepdims=True), 1e-12)
    fac_wn = fac_w / norm

    per_core = {"xT": [], "hT": [], "epsT": []}
    for c in range(N_CORES):
        rows = slice(c * B_CORE, (c + 1) * B_CORE)
        per_core["xT"].append(np.ascontiguousarray(input[rows].T))
        per_core["hT"].append(np.ascontiguousarray(h_0[rows].T))
        per_core["epsT"].append(np.ascontiguousarray(eps[rows].T))

    weights = {
        "Wci": np.ascontiguousarray(con_w_ih.T),
        "Wczr": np.ascontiguousarray(con_w_hh[:2 * CON].T),
        "Wcn": np.ascontiguousarray(con_w_hh[2 * CON:].T),
        "Wco": np.ascontiguousarray(co_w.T),
        "Wgi": np.ascontiguousarray(gen_w_ih.T),
        "Wgzr": np.ascontiguousarray(gen_w_hh[:2 * GEN].T),
        "Wgn": np.ascontiguousarray(gen_w_hh[2 * GEN:].T),
        "Wfac": np.ascontiguousarray(fac_wn.T),
    }

    gen_b_ih, gen_b_hh, con_b_ih, con_b_hh, co_b = [
        np.asarray(b, f) for b in biases]
    has_bias = any(np.any(b) for b in (gen_b_ih, gen_b_hh, con_b_ih,
                                       con_b_hh, co_b))
    if has_bias:
        bc = con_b_ih + con_b_hh
        bg = gen_b_ih + gen_b_hh
        weights["Bczr"] = np.ascontiguousarray(bc[:512].reshape(4, 128).T)
        weights["Bcn"] = np.ascontiguousarray(bc[512:].reshape(2, 128).T)
        weights["Bgzr"] = np.ascontiguousarray(bg[:1024].reshape(8, 128).T)
        weights["Bgn"] = np.ascontiguousarray(bg[1024:].reshape(4, 128).T)
        weights["Bco"] = np.ascontiguousarray(
            np.stack([co_b[:64], 0.5 * co_b[64:]], axis=1))
    return per_core, weights, has_bias


def kernel(input, h_0, eps, gen_w_ih, gen_b_ih, gen_w_hh, gen_b_hh,
           con_w_ih, con_b_ih, con_w_hh, con_b_hh, co_w, co_b, fac_w):
    per_core, weights, has_bias = _prep_inputs(
        input, h_0, eps, gen_w_ih, gen_w_hh, con_w_ih, con_w_hh, co_w, fac_w,
        (gen_b_ih, gen_b_hh, con_b_ih, con_b_hh, co_b))

    r = _get_runner(has_bias)
    jax = r.jax

    args = []
    for name in r.in_names:
        if name in per_core:
            a = np.concatenate(per_core[name], axis=0)
        else:
            a = np.concatenate([weights[name]] * N_CORES, axis=0)
        args.append(jax.device_put(a, r.sharding))
    for z in r.zero_outs:
        args.append(jax.device_put(
            np.zeros((N_CORES * z.shape[0], *z.shape[1:]), z.dtype),
            r.sharding))

    outs = jax.block_until_ready(r.fn(*args))
    outT_all = np.asarray(outs[r.out_names.index("outT")])
    outT_all = outT_all.reshape(N_CORES, H_DIM, B_CORE)

    out = np.empty((BATCH, H_DIM), np.float32)
    for c in range(N_CORES):
        out[c * B_CORE:(c + 1) * B_CORE] = outT_all[c].T
    return out



# revision 30
# speedup vs baseline: 2.9265x; 2.9265x over previous
"""Trainium2 Bass kernel for nn_DecoderCell (LFADS-style decoder cell).

Strategy: pure data parallel over 8 NeuronCores (batch 32768 -> 4096/core),
feature-major [feat, batch] on device. Per-core loop over 4 PAIRS of batch
tiles (N=512 each, pair width 1024 for DMA/elementwise), phased (all-con,
all-co, all-gen) so the Act engine loads the sigmoid/tanh table and the exp
table once each per pass. A zero-weight DoubleRow matmul gates each co-psum
on the last con tanh so the scheduler cannot interleave exp into the
sigmoid phases.

Precision plan (validated against the reference in numpy, rel ~1.4e-2 vs
the 2e-2 gate): GRU h- and (r*h)-matmuls run as fp8e4m3 DoubleRow pairs
(0.5 cyc/row on the PE); the x-path, co_linear and factor matmuls run in
bf16 (1 cyc/row); PSUM accumulation is fp32 and activations are f32-in /
bf16-out. DMA I/O is bf16 (+ an extra fp8 copy of h for the matmuls),
tile-pair-major in dram so every stream DMA is one 128-descriptor
contiguous-per-partition transfer. Outputs are written bf16 and upcast on
the host; the +-5 clip is applied host-side (only |h_0|>5 elements can
clip, and their downstream effect is within tolerance).
"""

import sys

sys.path.insert(0, "/opt/trn_rl_repo")

import numpy as np

import concourse.bacc as bacc
import concourse.tile as tile
import concourse.mybir as mybir

N_CORES = 8
BATCH = 32768
B_CORE = BATCH // N_CORES  # 4096
NT = 512                   # batch tile (free dim per matmul / PSUM bank)
NTILES = B_CORE // NT      # 8
NP = NTILES // 2           # 4 tile-pairs
PW = 2 * NT                # 1024 pair width

GEN, CON, CO, FAC, CIE, EXT = 512, 256, 64, 128, 128, 16
X_DIM = 2 * CIE + EXT      # 272
H_DIM = GEN + CON + 3 * CO + EXT + FAC  # 1104
CLIP = 5.0

F32 = mybir.dt.float32
BF = mybir.dt.bfloat16
F8 = mybir.dt.float8e4
AF = mybir.ActivationFunctionType
ALU = mybir.AluOpType
DR = mybir.MatmulPerfMode.DoubleRow

KNOBS = {
    "io_bufs": 2,
    "act_bufs": 2,
    "p2_bufs": 3,
    "p1_bufs": 2,
    "hc_bufs": 1,
    "gix_bufs": 1,
    "gate_exp": True,     # zero-weight matmul gating co psum on last con tanh
    "gate_a": True,       # gate next rep's first con zr groups on last gix
    # engine for each elementwise op: 'v' = DVE, 'g' = Pool/gpsimd
    "con_rh": "v",
    "con_d": "g",
    "con_zd": "v",
    "con_nd": "v",
    "gen_rh0": "v",
    "gen_rh1": "g",
    "gen_d0": "g",
    "gen_d1": "g",
    "gen_zd0": "v",
    "gen_zd1": "v",
    "gen_nd0": "v",
    "gen_nd1": "v",
}


def ts(i, s):
    return slice(i * s, (i + 1) * s)


def build_program(has_bias: bool, repeat: int = 1, num_devices: int = N_CORES):
    nc = bacc.Bacc("TRN2", target_bir_lowering=False, debug=False,
                   num_devices=num_devices)

    # ---- per-core streaming inputs, tile-pair-major [NP, P, flat] ----
    conxd = nc.dram_tensor("conxd", [NP, 128, 3 * PW], BF,
                           kind="ExternalInput")
    hc16d = nc.dram_tensor("hc16d", [NP, 128, 2 * PW], BF,
                           kind="ExternalInput")
    hc8d = nc.dram_tensor("hc8d", [NP, 128, 2 * PW], F8,
                          kind="ExternalInput")
    hg16d = nc.dram_tensor("hg16d", [NP, 128, 4 * PW], BF,
                           kind="ExternalInput")
    hg8d = nc.dram_tensor("hg8d", [NP, 128, 4 * PW], F8,
                          kind="ExternalInput")
    extd = nc.dram_tensor("extd", [NP, 16, PW], BF, kind="ExternalInput")
    epsd = nc.dram_tensor("epsd", [NP, 64, PW], BF, kind="ExternalInput")
    # weights, pre-packed on host into the SBUF layouts
    wcixd = nc.dram_tensor("wcixd", [128, 3, 768], BF, kind="ExternalInput")
    wczrd = nc.dram_tensor("wczrd", [128, 2, 512], F8, kind="ExternalInput")
    wcnd = nc.dram_tensor("wcnd", [128, 2, 256], F8, kind="ExternalInput")
    wcod = nc.dram_tensor("wcod", [128, 2, 128], BF, kind="ExternalInput")
    wgixd = nc.dram_tensor("wgixd", [80, 1, 1536], BF, kind="ExternalInput")
    wgzrd = nc.dram_tensor("wgzrd", [128, 4, 1024], F8, kind="ExternalInput")
    wgnd = nc.dram_tensor("wgnd", [128, 4, 512], F8, kind="ExternalInput")
    wfacd = nc.dram_tensor("wfacd", [128, 4, 128], BF, kind="ExternalInput")
    if has_bias:
        bczrd = nc.dram_tensor("bczrd", [128, 4], F32, kind="ExternalInput")
        bcnd = nc.dram_tensor("bcnd", [128, 2], F32, kind="ExternalInput")
        bcod = nc.dram_tensor("bcod", [64, 2], F32, kind="ExternalInput")
        bgzrd = nc.dram_tensor("bgzrd", [128, 8], F32, kind="ExternalInput")
        bgnd = nc.dram_tensor("bgnd", [128, 4], F32, kind="ExternalInput")

    # outputs, tile-pair-major
    od_gen = nc.dram_tensor("od_gen", [NP, 128, 4 * PW], BF,
                            kind="ExternalOutput")
    od_con = nc.dram_tensor("od_con", [NP, 128, 2 * PW], BF,
                            kind="ExternalOutput")
    od_msg = nc.dram_tensor("od_msg", [NP, 64, 2 * PW], BF,
                            kind="ExternalOutput")
    od_gi = nc.dram_tensor("od_gi", [NP, 64, PW], BF, kind="ExternalOutput")
    od_fac = nc.dram_tensor("od_fac", [NP, 128, PW], BF,
                            kind="ExternalOutput")

    with tile.TileContext(nc) as tc:
        with (
            tc.tile_pool(name="w", bufs=1) as wp,
            tc.tile_pool(name="io", bufs=KNOBS["io_bufs"]) as io,
            tc.tile_pool(name="act", bufs=KNOBS["act_bufs"]) as act,
            tc.tile_pool(name="hcp", bufs=1) as hcp,
            tc.tile_pool(name="gixp", bufs=1) as gixp,
            tc.tile_pool(name="p2", bufs=KNOBS["p2_bufs"], space="PSUM") as p2,
            tc.tile_pool(name="p1", bufs=KNOBS["p1_bufs"], space="PSUM") as p1,
        ):
            ENG = {"v": nc.vector, "g": nc.gpsimd}

            def wload(dram, shape, dt, tag):
                t = wp.tile(shape, dt, tag=tag)
                nc.sync.dma_start(out=t, in_=dram[:, :, :])
                return t

            Wcix = wload(wcixd, [128, 3, 768], BF, "Wcix")
            Wczr = wload(wczrd, [128, 2, 512], F8, "Wczr")
            Wcn = wload(wcnd, [128, 2, 256], F8, "Wcn")
            Wco = wload(wcod, [128, 2, 128], BF, "Wco")
            Wgix = wload(wgixd, [80, 1, 1536], BF, "Wgix")
            Wgzr = wload(wgzrd, [128, 4, 1024], F8, "Wgzr")
            Wgn = wload(wgnd, [128, 4, 512], F8, "Wgn")
            Wfac = wload(wfacd, [128, 4, 128], BF, "Wfac")
            if KNOBS["gate_exp"] or KNOBS["gate_a"]:
                Zw = wp.tile([128, 128], BF, tag="Zw")
                nc.vector.memset(Zw, 0)
            if has_bias:
                Bczr = wp.tile([128, 4], F32, tag="Bczr")
                nc.sync.dma_start(out=Bczr, in_=bczrd[:, :])
                Bcn = wp.tile([128, 2], F32, tag="Bcn")
                nc.sync.dma_start(out=Bcn, in_=bcnd[:, :])
                Bco = wp.tile([64, 2], F32, tag="Bco")
                nc.sync.dma_start(out=Bco, in_=bcod[:, :])
                Bgzr = wp.tile([128, 8], F32, tag="Bgzr")
                nc.sync.dma_start(out=Bgzr, in_=bgzrd[:, :])
                Bgn = wp.tile([128, 4], F32, tag="Bgn")
                nc.sync.dma_start(out=Bgn, in_=bgnd[:, :])

            def act_write(dst_tile, c0, t, psum, nch, func, bias_tile,
                          bias_c0):
                # dst_tile: [128, C, 2, NT]; writes chunks c0:c0+nch, tile t
                if has_bias:
                    for c in range(nch):
                        nc.scalar.activation(
                            dst_tile[:, c0 + c, t, :], psum[:, c, :], func,
                            bias=bias_tile[:, bias_c0 + c:bias_c0 + c + 1])
                else:
                    nc.scalar.activation(dst_tile[:, c0:c0 + nch, t, :],
                                         psum[:, 0:nch, :], func)

            def split2(op_name, dst, a, b, k0, k1, nch):
                """Tensor-tensor op over [128, nch, 2, 512] pair tensors,
                split into chunk halves across engines; one instr if same."""
                h = nch // 2
                if k0 == k1:
                    getattr(ENG[k0], op_name)(
                        dst[:, 0:nch, :, :], a[:, 0:nch, :, :],
                        b[:, 0:nch, :, :])
                else:
                    getattr(ENG[k0], op_name)(
                        dst[:, 0:h, :, :], a[:, 0:h, :, :], b[:, 0:h, :, :])
                    getattr(ENG[k1], op_name)(
                        dst[:, h:nch, :, :], a[:, h:nch, :, :],
                        b[:, h:nch, :, :])

            # ---------------- stages (per tile-pair) ----------------
            def con_fetch(p):
                cx = io.tile([128, 3, 2, NT], BF, tag="cx", bufs=3)
                nc.sync.dma_start(
                    out=cx,
                    in_=conxd[p].rearrange("p (c t n) -> p c t n", c=3, t=2))
                h16 = hcp.tile([128, 2, 2, NT], BF, tag=f"hc{p}",
                               bufs=KNOBS["hc_bufs"])
                nc.sync.dma_start(
                    out=h16,
                    in_=hc16d[p].rearrange("p (c t n) -> p c t n", c=2, t=2))
                h8 = io.tile([128, 2, 2, NT], F8, tag="hc8", bufs=3)
                nc.sync.dma_start(
                    out=h8,
                    in_=hc8d[p].rearrange("p (c t n) -> p c t n", c=2, t=2))
                return cx, h16, h8

            def con_stage(p, fetched, gate_src=None):
                cx, h16, h8 = fetched
                zr = act.tile([128, 4, 2, NT], BF, tag="zr_c")
                for t in range(2):
                    for g in range(2):
                        pz = p2.tile([128, 2, NT], F32, tag="p2")
                        for mi in range(2):
                            m = 2 * g + mi
                            for k in range(3):
                                nc.tensor.matmul(pz[:, mi, :],
                                                 Wcix[:, k, ts(m, 128)],
                                                 cx[:, k, t, :],
                                                 start=(k == 0), stop=False)
                            nc.tensor.matmul(pz[:, mi, :],
                                             Wczr[:, :, ts(m, 128)],
                                             h8[:, :, t, :],
                                             start=False,
                                             stop=(gate_src is None
                                                   or mi == 1),
                                             perf_mode=DR)
                            if gate_src is not None and mi == 0:
                                # gate next rep's sigmoids on the previous
                                # rep's exp phase (via its last gix tile);
                                # the act reads both mi groups, so gating
                                # mi=0 gates the activation transitively
                                nc.tensor.matmul(pz[:, mi, :], Zw[0:64, :],
                                                 gate_src,
                                                 start=False, stop=True)
                        act_write(zr, 2 * g, t, pz, 2, AF.Sigmoid,
                                  has_bias and Bczr, 2 * g)

                rh = act.tile([128, 2, 2, NT], F8, tag="rh8")
                for t in range(2):
                    ENG[KNOBS["con_rh"]].tensor_mul(rh[:, :, t, :],
                                                    zr[:, 2:4, t, :],
                                                    h16[:, :, t, :])

                n_t = act.tile([128, 2, 2, NT], BF, tag="n_c")
                for t in range(2):
                    pn = p2.tile([128, 2, NT], F32, tag="p2")
                    for mi in range(2):
                        for k in range(3):
                            nc.tensor.matmul(pn[:, mi, :],
                                             Wcix[:, k, ts(4 + mi, 128)],
                                             cx[:, k, t, :],
                                             start=(k == 0), stop=False)
                        nc.tensor.matmul(pn[:, mi, :], Wcn[:, :, ts(mi, 128)],
                                         rh[:, :, t, :],
                                         start=False, stop=True, perf_mode=DR)
                    act_write(n_t, 0, t, pn, 2, AF.Tanh, has_bias and Bcn, 0)

                # h' = n + z*(h - n), in place on h16 (clip on host)
                ENG[KNOBS["con_d"]].tensor_sub(h16, h16[:, :, :, :],
                                               n_t[:, :, :, :])
                ENG[KNOBS["con_zd"]].tensor_mul(h16, zr[:, 0:2, :, :],
                                                h16[:, :, :, :])
                ENG[KNOBS["con_nd"]].tensor_add(h16, n_t[:, :, :, :],
                                                h16[:, :, :, :])
                nc.sync.dma_start(
                    out=od_con[p].rearrange("p (c t n) -> p c t n", c=2, t=2),
                    in_=h16)
                return h16, n_t

            def co_stage(p, h16, gate_nt, epsall):
                msg = io.tile([64, 2, 2, NT], BF, tag="msg")
                for t in range(2):
                    P = p2.tile([128, 2, NT], F32, tag="p2")
                    pm = P[0:64, 0, :]
                    pv = P[0:64, 1, :]
                    for k in range(2):
                        nc.tensor.matmul(pm, Wco[:, k, 0:64],
                                         h16[:, k, t, :],
                                         start=(k == 0), stop=(k == 1))
                    for k in range(2):
                        nc.tensor.matmul(pv, Wco[:, k, 64:128],
                                         h16[:, k, t, :],
                                         start=(k == 0),
                                         stop=(k == 1 and gate_nt is None))
                    if gate_nt is not None:
                        # zero-weight matmul: data-gates this psum (and
                        # hence the exp below) on the last con-phase tanh,
                        # so exps cannot interleave into the sigmoid phase
                        nc.tensor.matmul(pv, Zw[:, 0:64],
                                         gate_nt[:, 0, :, 0:256],
                                         start=False, stop=True)
                    if has_bias:
                        nc.scalar.activation(msg[:, 0, t, :], pm,
                                             AF.Identity, bias=Bco[:, 0:1])
                        nc.scalar.activation(msg[:, 1, t, :], pv,
                                             AF.Exp, scale=0.5,
                                             bias=Bco[:, 1:2])
                    else:
                        nc.vector.tensor_copy(msg[:, 0, t, :], pm)
                        nc.scalar.activation(msg[:, 1, t, :], pv,
                                             AF.Exp, scale=0.5)
                ep = epsall[:, p, :, :]
                nc.vector.tensor_mul(ep, msg[:, 1, :, :], ep)
                gix = gixp.tile([80, 2, NT], BF, tag=f"gix{p}",
                                bufs=KNOBS["gix_bufs"])
                nc.vector.tensor_add(gix[0:64, :, :], msg[:, 0, :, :], ep)
                nc.sync.dma_start(out=gix[64:80, :, :], in_=extd[p].rearrange(
                    "p (t n) -> p t n", t=2))
                nc.sync.dma_start(
                    out=od_msg[p].rearrange("p (c t n) -> p c t n", c=2, t=2),
                    in_=msg)
                nc.sync.dma_start(
                    out=od_gi[p].rearrange("p (t n) -> p t n", t=2),
                    in_=gix[0:64, :, :])
                return gix

            def gen_fetch(p):
                hg16 = io.tile([128, 4, 2, NT], BF, tag="hg16", bufs=3)
                nc.sync.dma_start(
                    out=hg16,
                    in_=hg16d[p].rearrange("p (c t n) -> p c t n", c=4, t=2))
                hg8 = io.tile([128, 4, 2, NT], F8, tag="hg8", bufs=3)
                nc.sync.dma_start(
                    out=hg8,
                    in_=hg8d[p].rearrange("p (c t n) -> p c t n", c=4, t=2))
                return hg16, hg8

            def gen_stage(p, gix, fetched):
                hg16, hg8 = fetched
                zg = act.tile([128, 4, 2, NT], BF, tag="zg")
                rg16 = act.tile([128, 4, 2, NT], BF, tag="rg16")
                for t in range(2):
                    for g in range(4):
                        pz = p2.tile([128, 2, NT], F32, tag="p2")
                        for mi in range(2):
                            m = 2 * g + mi
                            nc.tensor.matmul(pz[:, mi, :],
                                             Wgzr[:, 0:2, ts(m, 128)],
                                             hg8[:, 0:2, t, :],
                                             start=True, stop=False,
                                             perf_mode=DR)
                            nc.tensor.matmul(pz[:, mi, :],
                                             Wgzr[:, 2:4, ts(m, 128)],
                                             hg8[:, 2:4, t, :],
                                             start=False, stop=False,
                                             perf_mode=DR)
                            nc.tensor.matmul(pz[:, mi, :],
                                             Wgix[:, 0, ts(m, 128)],
                                             gix[:, t, :],
                                             start=False, stop=True)
                        dst, c0 = ((zg, 2 * g) if g < 2
                                   else (rg16, 2 * (g - 2)))
                        act_write(dst, c0, t, pz, 2, AF.Sigmoid,
                                  has_bias and Bgzr, 2 * g)

                rg8 = act.tile([128, 4, 2, NT], F8, tag="rg8", bufs=1)
                split2("tensor_mul", rg8, rg16, hg16,
                       KNOBS["gen_rh0"], KNOBS["gen_rh1"], 4)

                ng = act.tile([128, 4, 2, NT], BF, tag="ng", bufs=1)
                for t in range(2):
                    for g in range(2):
                        pn = p2.tile([128, 2, NT], F32, tag="p2")
                        for mi in range(2):
                            m = 2 * g + mi
                            nc.tensor.matmul(pn[:, mi, :],
                                             Wgix[:, 0, ts(8 + m, 128)],
                                             gix[:, t, :],
                                             start=True, stop=False)
                            nc.tensor.matmul(pn[:, mi, :],
                                             Wgn[:, 0:2, ts(m, 128)],
                                             rg8[:, 0:2, t, :],
                                             start=False, stop=False,
                                             perf_mode=DR)
                            nc.tensor.matmul(pn[:, mi, :],
                                             Wgn[:, 2:4, ts(m, 128)],
                                             rg8[:, 2:4, t, :],
                                             start=False, stop=True,
                                             perf_mode=DR)
                        act_write(ng, 2 * g, t, pn, 2, AF.Tanh,
                                  has_bias and Bgn, 2 * g)

                split2("tensor_sub", hg16, hg16, ng,
                       KNOBS["gen_d0"], KNOBS["gen_d1"], 4)
                split2("tensor_mul", hg16, zg, hg16,
                       KNOBS["gen_zd0"], KNOBS["gen_zd1"], 4)
                split2("tensor_add", hg16, ng, hg16,
                       KNOBS["gen_nd0"], KNOBS["gen_nd1"], 4)
                nc.sync.dma_start(
                    out=od_gen[p].rearrange("p (c t n) -> p c t n", c=4, t=2),
                    in_=hg16)

                fc = io.tile([128, 2, NT], BF, tag="fc")
                for t in range(2):
                    pf = p1.tile([128, NT], F32, tag="p1")
                    for k in range(4):
                        nc.tensor.matmul(pf, Wfac[:, k, :], hg16[:, k, t, :],
                                         start=(k == 0), stop=(k == 3))
                    nc.vector.tensor_copy(fc[:, t, :], pf)
                nc.sync.dma_start(
                    out=od_fac[p].rearrange("p (t n) -> p t n", t=2), in_=fc)

            prev_gix = None
            fA = None
            for _rep in range(repeat):
                if fA is None:
                    fA = [con_fetch(0), con_fetch(1)]
                epsall = io.tile([64, NP, 2, NT], BF, tag="epsall")
                nc.sync.dma_start(
                    out=epsall,
                    in_=epsd[:, :, :].rearrange("t p (u n) -> p t u n", u=2))
                hs = []
                for p in range(NP):
                    if p + 2 < NP:
                        fA.append(con_fetch(p + 2))
                    gs = (prev_gix[0:64, :, 0:256]
                          if (p == 0 and prev_gix is not None
                              and KNOBS["gate_a"]) else None)
                    hs.append(con_stage(p, fA[p], gs))
                fC = [gen_fetch(0), gen_fetch(1)]
                gate_nt = hs[NP - 1][1] if KNOBS["gate_exp"] else None
                gixs = [co_stage(p, hs[p][0], gate_nt, epsall)
                        for p in range(NP)]
                nextA = []
                for p in range(NP):
                    if p + 2 < NP:
                        fC.append(gen_fetch(p + 2))
                    elif _rep + 1 < repeat:
                        nextA.append(con_fetch(p + 2 - NP))
                    gen_stage(p, gixs[p], fC[p])
                prev_gix = gixs[NP - 1]
                fA = nextA if nextA else None

    nc.compile()
    return nc


# ---------------------------------------------------------------------------
# host-side prep
# ---------------------------------------------------------------------------
def _enc_stream(arr_t, dt):
    """[rows, B_CORE] feature-major -> tile-pair-major
    [NP, 128orP, c*2*NT] with rows = c*P + p, cols = pair*PW + off."""
    rows, B = arr_t.shape
    P = min(rows, 128)
    c = rows // P
    assert c * P == rows
    a = arr_t.reshape(c, P, NP, PW)        # [c, p, pair, t*n]
    a = a.transpose(2, 1, 0, 3)            # [pair, p, c, t*n]
    return np.ascontiguousarray(a.reshape(NP, P, c * PW).astype(dt))


def _dec_stream(a, rows):
    """Inverse of _enc_stream: [NP, P, c*PW] f32 -> [rows, B_CORE]."""
    NP_, P, flat = a.shape
    c = rows // P
    a = a.reshape(NP_, P, c, PW).transpose(2, 1, 0, 3)  # [c, p, pair, t*n]
    return a.reshape(rows, NP_ * PW)


def _prep_inputs(input, h_0, eps, gen_w_ih, gen_w_hh, con_w_ih, con_w_hh,
                 co_w, fac_w, biases):
    import ml_dtypes
    f = np.float32
    BF_NP = ml_dtypes.bfloat16
    F8_NP = ml_dtypes.float8_e4m3

    input = np.asarray(input, f)
    h_0 = np.asarray(h_0, f)
    eps = np.asarray(eps, f)
    gen_w_ih = np.asarray(gen_w_ih, f)
    gen_w_hh = np.asarray(gen_w_hh, f)
    con_w_ih = np.asarray(con_w_ih, f)
    con_w_hh = np.asarray(con_w_hh, f)
    co_w = np.asarray(co_w, f)
    fac_w = np.asarray(fac_w, f)

    norm = np.maximum(np.linalg.norm(fac_w, axis=1, keepdims=True), 1e-12)
    fac_wn = fac_w / norm

    def drpack(w_t, dt):
        # w_t: [K, M] feature-major weight -> [128, K//128, M],
        # element [p, c, m] = w_t[c*128 + p, m]
        K, M = w_t.shape
        return np.ascontiguousarray(
            w_t.reshape(K // 128, 128, M).transpose(1, 0, 2).astype(dt))

    H_FAC = GEN + CON + 3 * CO + EXT
    per_core = {}
    for c in range(N_CORES):
        rows = slice(c * B_CORE, (c + 1) * B_CORE)
        xT = input[rows].T
        hT = h_0[rows].T
        conx = np.concatenate([xT[0:256], hT[H_FAC:H_FAC + 128]], axis=0)
        hcon = hT[GEN:GEN + CON]
        hgen = hT[0:GEN]
        per_core.setdefault("conxd", []).append(_enc_stream(conx, BF_NP))
        per_core.setdefault("hc16d", []).append(_enc_stream(hcon, BF_NP))
        per_core.setdefault("hc8d", []).append(_enc_stream(hcon, F8_NP))
        per_core.setdefault("hg16d", []).append(_enc_stream(hgen, BF_NP))
        per_core.setdefault("hg8d", []).append(_enc_stream(hgen, F8_NP))
        per_core.setdefault("extd", []).append(
            _enc_stream(xT[256:272], BF_NP))
        per_core.setdefault("epsd", []).append(
            _enc_stream(eps[rows].T, BF_NP))

    weights = {
        "wcixd": drpack(con_w_ih.T, BF_NP),              # [128, 3, 768]
        "wczrd": drpack(con_w_hh[:2 * CON].T, F8_NP),    # [128, 2, 512]
        "wcnd": drpack(con_w_hh[2 * CON:].T, F8_NP),     # [128, 2, 256]
        "wcod": drpack(co_w.T, BF_NP),                   # [128, 2, 128]
        "wgixd": np.ascontiguousarray(
            gen_w_ih.T.astype(BF_NP)).reshape(80, 1, 1536),
        "wgzrd": drpack(gen_w_hh[:2 * GEN].T, F8_NP),    # [128, 4, 1024]
        "wgnd": drpack(gen_w_hh[2 * GEN:].T, F8_NP),     # [128, 4, 512]
        "wfacd": drpack(fac_wn.T, BF_NP),                # [128, 4, 128]
    }

    gen_b_ih, gen_b_hh, con_b_ih, con_b_hh, co_b = [
        np.asarray(b, f) for b in biases]
    has_bias = any(np.any(b) for b in (gen_b_ih, gen_b_hh, con_b_ih,
                                       con_b_hh, co_b))
    if has_bias:
        bc = con_b_ih + con_b_hh
        bg = gen_b_ih + gen_b_hh
        weights["bczrd"] = np.ascontiguousarray(bc[:512].reshape(4, 128).T)
        weights["bcnd"] = np.ascontiguousarray(bc[512:].reshape(2, 128).T)
        weights["bgzrd"] = np.ascontiguousarray(bg[:1024].reshape(8, 128).T)
        weights["bgnd"] = np.ascontiguousarray(bg[1024:].reshape(4, 128).T)
        weights["bcod"] = np.ascontiguousarray(
            np.stack([co_b[:64], 0.5 * co_b[64:]], axis=1))
    return per_core, weights, has_bias


def _assemble_core(outs_c, input_rows):
    """outs_c: dict name -> per-core output array (f32). Returns
    [B_CORE, H_DIM] f32 with host clip + ext passthrough."""
    out = np.empty((B_CORE, H_DIM), np.float32)
    out[:, 0:512] = _dec_stream(outs_c["od_gen"], 512).T
    out[:, 512:768] = _dec_stream(outs_c["od_con"], 256).T
    ms = _dec_stream(outs_c["od_msg"], 128)
    out[:, 768:832] = ms[0:64].T
    out[:, 832:896] = ms[64:128].T
    out[:, 896:960] = _dec_stream(outs_c["od_gi"], 64).T
    out[:, 976:1104] = _dec_stream(outs_c["od_fac"], 128).T
    np.clip(out[:, 0:768], -CLIP, CLIP, out=out[:, 0:768])
    out[:, 960:976] = input_rows[:, 256:272]
    return out


# ---------------------------------------------------------------------------
# host-side runner (cached per process)
# ---------------------------------------------------------------------------
_CACHE = {}


def _get_runner(has_bias):
    key = has_bias
    if key not in _CACHE:
        nc = build_program(has_bias)
        _CACHE[key] = _make_runner(nc)
    return _CACHE[key]


def _make_runner(nc):
    import jax
    from jax.sharding import Mesh, PartitionSpec, NamedSharding
    from jax.experimental.shard_map import shard_map
    from concourse.bass2jax import (_bass_exec_p, install_neuronx_cc_hook,
                                    partition_id_tensor)

    install_neuronx_cc_hook()
    partition_name = (nc.partition_id_tensor.name
                      if nc.partition_id_tensor else None)
    in_names, out_names, out_avals, zero_outs = [], [], [], []
    for alloc in nc.m.functions[0].allocations:
        if not isinstance(alloc, mybir.MemoryLocationSet):
            continue
        name = alloc.memorylocations[0].name
        if alloc.kind == "ExternalInput":
            if name != partition_name:
                in_names.append(name)
        elif alloc.kind == "ExternalOutput":
            shape = tuple(alloc.tensor_shape)
            dtype = mybir.dt.np(alloc.dtype)
            out_names.append(name)
            out_avals.append(jax.core.ShapedArray(shape, dtype))
            zero_outs.append(np.zeros(shape, dtype))
    all_in = in_names + out_names
    if partition_name is not None:
        all_in.append(partition_name)

    def _body(*args):
        operands = list(args)
        if partition_name is not None:
            operands.append(partition_id_tensor())
        return tuple(_bass_exec_p.bind(
            *operands, out_avals=tuple(out_avals), in_names=tuple(all_in),
            out_names=tuple(out_names),
            lowering_input_output_aliases=(),
            sim_require_finite=True, sim_require_nnan=True, nc=nc))

    devices = jax.devices()[:N_CORES]
    mesh = Mesh(np.asarray(devices), ("core",))
    nin = len(in_names)
    fn = jax.jit(
        shard_map(_body, mesh=mesh,
                  in_specs=(PartitionSpec("core"),) * (nin + len(out_names)),
                  out_specs=(PartitionSpec("core"),) * len(out_names),
                  check_rep=False),
        keep_unused=True)
    sharding = NamedSharding(mesh, PartitionSpec("core"))

    class R:
        pass

    r = R()
    r.jax = jax
    r.fn = fn
    r.sharding = sharding
    r.in_names = in_names
    r.out_names = out_names
    r.out_avals = out_avals
    r.zero_outs = zero_outs
    return r


def kernel(input, h_0, eps, gen_w_ih, gen_b_ih, gen_w_hh, gen_b_hh,
           con_w_ih, con_b_ih, con_w_hh, con_b_hh, co_w, co_b, fac_w):
    input = np.asarray(input, np.float32)
    per_core, weights, has_bias = _prep_inputs(
        input, h_0, eps, gen_w_ih, gen_w_hh, con_w_ih, con_w_hh, co_w, fac_w,
        (gen_b_ih, gen_b_hh, con_b_ih, con_b_hh, co_b))

    r = _get_runner(has_bias)
    jax = r.jax

    args = []
    for name in r.in_names:
        if name in per_core:
            a = np.concatenate(per_core[name], axis=0)
        else:
            a = np.concatenate([weights[name]] * N_CORES, axis=0)
        args.append(jax.device_put(a, r.sharding))
    for z in r.zero_outs:
        args.append(jax.device_put(
            np.zeros((N_CORES * z.shape[0], *z.shape[1:]), z.dtype),
            r.sharding))

    outs = jax.block_until_ready(r.fn(*args))
    out = np.empty((BATCH, H_DIM), np.float32)
    for c in range(N_CORES):
        rows = slice(c * B_CORE, (c + 1) * B_CORE)
        outs_c = {}
        for i, name in enumerate(r.out_names):
            a = np.asarray(outs[i]).astype(np.float32)
            pershard = a.shape[0] // N_CORES
            outs_c[name] = a[c * pershard:(c + 1) * pershard]
        out[rows] = _assemble_core(outs_c, input[rows])
    return out
